# revision 1
# baseline (speedup 1.0000x reference)
"""DiagSSMBlock Trainium2 kernel.

h_t = sum_{k=0..t} a^k * (B^T x_{t-k})  ==  h_t = a * h_{t-1} + s_t, s = B^T x^T.

Strategy: shard T across the 8 cores (1024 steps each + 32-step halo; |a| <=
sqrt(2/1024) ~ 0.044 so a^32 < 1e-43 == 0 in fp32, making slabs exactly
independent).  Host passes x pre-transposed ([H, T_slab]) so the tensor engine
can contract over H with no on-chip transposes; the scan output is returned
channel-major [H, T_slab] and transposed back on host.

Per core: DMA B + xT slab -> 8x K-chunked fp32r matmul accumulation into PSUM
(3 chunks of 352 time-cols) -> tensor_tensor_scan (the SSM recurrence) per
128-channel group -> DMA out.  Dummy warm-up matmuls run during the input DMA
ramp so the PE HAM clock-gate reaches 2.4 GHz before the real matmuls start;
DMA issue is spread across the two HWDGE engines (sync + scalar).

Scheduling note: the a-broadcast tiles are built with gpsimd tensor_scalar,
which is slow (~5us each) -- measured FASTER end-to-end than building them on
DVE, because the slow drip of a_bc completions paces scan start (and hence
psum-slot recycling and output stores) to roughly match DMA supply, keeping
the DVE out of the PE's way during the input-bound phase.
"""

import sys

if "/opt/trn_rl_repo" not in sys.path:
    sys.path.insert(0, "/opt/trn_rl_repo")

import numpy as np

T, H = 8192, 1024
NC = 8
P = 128
T_LOC = T // NC            # 1024 output timesteps per core
HALO = 32                  # scan warmup; a^32 == 0 in fp32
W = T_LOC + HALO           # 1056
CH = 352                   # psum chunk width (3 chunks of 352 = 1056)
NCHUNK = W // CH
KQ = H // P                # 8 contraction chunks
G = H // P                 # 8 channel groups
N_WARM = 12                # dummy matmuls to lift the HAM clock gate

MM_DTYPE = "float32r"      # matmul operand dtype: "float32" (4 cyc/row) or
                           # "float32r" (1 cyc/row at N>=256)

_state = {}


def _build_nc():
    import concourse.tile as tile
    from concourse import bacc, mybir

    mm_dt = getattr(mybir.dt, MM_DTYPE)
    f32 = mybir.dt.float32

    nc = bacc.Bacc("TRN2", target_bir_lowering=False, debug=False, num_devices=NC)
    xt_e = nc.dram_tensor("xt", [H, W], mm_dt, kind="ExternalInput").ap()
    b_e = nc.dram_tensor("b", [H, H], mm_dt, kind="ExternalInput").ap()
    av_e = nc.dram_tensor("av", [P, G], f32, kind="ExternalInput").ap()
    out_e = nc.dram_tensor("out", [H, T_LOC], f32, kind="ExternalOutput").ap()
    flush_e = nc.dram_tensor("warm_flush", [P, 1], f32).ap()

    with tile.TileContext(nc) as tc:
        with (
            tc.tile_pool(name="consts", bufs=1) as consts,
            tc.tile_pool(name="bpool", bufs=1) as bpool,
            tc.tile_pool(name="xpool", bufs=1) as xpool,
            tc.tile_pool(name="hpool", bufs=1) as hpool,
            tc.tile_pool(name="pspool", bufs=6, space="PSUM") as pspool,
            tc.tile_pool(name="warmps", bufs=1, space="PSUM") as warmps,
        ):
            # PE warm-up: dummy fp32 matmuls on a zeroed scratch tile, gated
            # only on a gpsimd memset, so the HAM clock-gate lifts to 2.4 GHz
            # during the input-DMA ramp.
            warm_sb = consts.tile([P, P], f32, tag="warm")
            nc.gpsimd.memset(warm_sb[:], 0.0)
            wps = warmps.tile([P, P], f32)
            for i in range(N_WARM):
                nc.tensor.matmul(
                    wps[:],
                    warm_sb[:],
                    warm_sb[:],
                    start=(i == 0),
                    stop=(i == N_WARM - 1),
                )
            flush_sb = consts.tile([P, 1], f32, tag="flush")
            nc.vector.tensor_copy(flush_sb[:], wps[:, 0:1])
            nc.gpsimd.dma_start(flush_e[:], flush_sb[:])

            # a broadcast tiles (built on gpsimd; see module docstring)
            av_sb = consts.tile([P, G], f32, tag="av")
            nc.sync.dma_start(av_sb[:], av_e[:])
            a_bc = []
            for g in range(G):
                t = consts.tile([P, CH], f32, tag=f"abc{g}")
                nc.gpsimd.memset(t[:], 1.0)
                nc.gpsimd.tensor_scalar_mul(t[:], t[:], av_sb[:, g : g + 1])
                a_bc.append(t)

            # Input loads.  sync carries xt chunks 0 and 2; scalar carries the
            # group-0 b tiles (fine-grained for a fast start), xt chunk 1, the
            # rest of b, and the output stores.
            b_g0 = []
            for kq in range(KQ):
                bt = bpool.tile([P, P], mm_dt, tag=f"bg0_{kq}")
                nc.scalar.dma_start(
                    bt[:], b_e[kq * P : (kq + 1) * P, 0:P]
                )
                b_g0.append(bt)
            xt_sb = [[None] * NCHUNK for _ in range(KQ)]
            for ni in range(NCHUNK):
                eng = nc.scalar if ni == 1 else nc.sync
                n0 = ni * CH
                for kq in range(KQ):
                    xtile = xpool.tile([P, CH], mm_dt, tag=f"x{kq}_{ni}")
                    eng.dma_start(
                        xtile[:], xt_e[kq * P : (kq + 1) * P, n0 : n0 + CH]
                    )
                    xt_sb[kq][ni] = xtile
            b_rest = []
            for kq in range(KQ):
                bt = bpool.tile([P, H - P], mm_dt, tag=f"br_{kq}")
                nc.scalar.dma_start(bt[:], b_e[kq * P : (kq + 1) * P, P:H])
                b_rest.append(bt)

            def b_slice(kq, g):
                if g == 0:
                    return b_g0[kq][:]
                return b_rest[kq][:, (g - 1) * P : g * P]

            for g in range(G):
                h_t = hpool.tile([P, W], f32, tag=f"h{g}")
                for ni in range(NCHUNK):
                    n0 = ni * CH
                    ps = pspool.tile([P, CH], f32)
                    for kq in range(KQ):
                        nc.tensor.matmul(
                            ps[:],
                            b_slice(kq, g),
                            xt_sb[kq][ni][:],
                            start=(kq == 0),
                            stop=(kq == KQ - 1),
                        )
                    init = 0.0 if ni == 0 else h_t[:, n0 - 1 : n0]
                    nc.vector.tensor_tensor_scan(
                        h_t[:, n0 : n0 + CH],
                        a_bc[g][:],
                        ps[:],
                        init,
                        op0=mybir.AluOpType.mult,
                        op1=mybir.AluOpType.add,
                    )
                    if g < 3:
                        # keep-warm fillers: occupy the PE during input-DMA
                        # stalls of the early phase so HAM stays at 2.4 GHz
                        for i in range(2):
                            nc.tensor.matmul(
                                wps[:],
                                warm_sb[:],
                                warm_sb[:],
                                start=(i == 0),
                                stop=(i == 1),
                            )
                nc.scalar.dma_start(out_e[g * P : (g + 1) * P, :], h_t[:, HALO:W])

    nc.compile()
    return nc


def _get_nc():
    if "nc" not in _state:
        _state["nc"] = _build_nc()
    return _state["nc"]


def _shard_inputs(x_seq, a_diag, b_mat):
    x = np.asarray(x_seq, dtype=np.float32)
    a = np.asarray(a_diag, dtype=np.float32)
    b = np.ascontiguousarray(np.asarray(b_mat, dtype=np.float32))
    x_pad = np.concatenate([np.zeros((HALO, H), np.float32), x], axis=0)
    xT = np.ascontiguousarray(x_pad.T)  # [H, T + HALO]
    av = np.ascontiguousarray(a.reshape(G, P).T)  # [P, G]
    in_maps = []
    for i in range(NC):
        in_maps.append(
            {
                "xt": np.ascontiguousarray(xT[:, i * T_LOC : i * T_LOC + W]),
                "b": b,
                "av": av,
            }
        )
    return in_maps


def kernel(x_seq, a_diag, b_mat):
    from concourse.bass_utils import run_bass_kernel_spmd

    nc = _get_nc()
    in_maps = _shard_inputs(x_seq, a_diag, b_mat)
    res = run_bass_kernel_spmd(nc, in_maps, list(range(NC)))
    _state["last_result"] = res
    out = np.concatenate(
        [np.asarray(res.results[i]["out"]).T for i in range(NC)], axis=0
    )
    return out



# revision 3
# speedup vs baseline: 1.2190x; 1.2190x over previous
"""DiagSSMBlock Trainium2 kernel.

h_t = sum_{k=0..t} a^k * (B^T x_{t-k})  ==  h_t = a * h_{t-1} + s_t, s = B^T x^T.

Strategy: shard T across the 8 cores (1024 steps each + 8-step halo; |a| <=
sqrt(2/1024) ~ 0.044 so a^9 ~ 6e-13 -- far below the 2e-2 gate, making slabs
independent).  All matmul operands are bf16 (halves input DMA vs fp32; PE
streams bf16 at the same 1 col/cycle as fp32r; accumulation stays fp32 in
PSUM).  Host pre-lays-out every DRAM tensor so each DMA is contiguous per
partition.

Per core: s slab = B^T x^T computed as 8 channel groups x 3 time chunks of
344, accumulating 8 K-blocks per chunk into PSUM; the SSM recurrence runs as
tensor_tensor_scan on DVE (fp32 internal state, bf16 out); output stored
bf16 per channel group.

Loop order is chunk-column-outer (ni, then g) so the PE's data needs follow
DMA arrival order: chunk 0 of x plus the first b group unlock work ~2.5us in,
and each subsequent b group / x chunk lands well before the PE reaches it.
Warm-up matmuls run during the input DMA ramp to lift the HAM clock gate.
"""

import sys

if "/opt/trn_rl_repo" not in sys.path:
    sys.path.insert(0, "/opt/trn_rl_repo")

import numpy as np
import ml_dtypes

T, H = 8192, 1024
NC = 8
P = 128
T_LOC = T // NC            # 1024 output timesteps per core
HALO = 8                   # scan warmup; a^9 ~ 6e-13
W = T_LOC + HALO           # 1032
CH = 344                   # psum chunk width (3 chunks of 344 = 1032)
NCHUNK = W // CH           # 3
KQ = H // P                # 8 contraction blocks
G = H // P                 # 8 channel groups
N_WARM = 14                # dummy matmuls to lift the HAM clock gate

BF16 = ml_dtypes.bfloat16

_state = {}


def _build_nc():
    import concourse.tile as tile
    from concourse import bacc, mybir

    bf16 = mybir.dt.bfloat16
    f32 = mybir.dt.float32

    nc = bacc.Bacc("TRN2", target_bir_lowering=False, debug=False, num_devices=NC)
    # xt: chunk-major: [P, ni, kq, CH] flattened -> chunk ni is one contiguous
    # [P, KQ*CH] slab per partition.
    xt_e = nc.dram_tensor("xt", [P, NCHUNK * KQ * CH], bf16, kind="ExternalInput").ap()
    # b: group-major: [P, g, kq, 128] flattened -> group g is contiguous.
    b_e = nc.dram_tensor("b", [P, G * H], bf16, kind="ExternalInput").ap()
    av_e = nc.dram_tensor("av", [P, G], f32, kind="ExternalInput").ap()
    # out: [P, g, T_LOC] flattened, bf16.
    out_e = nc.dram_tensor("out", [P, G * T_LOC], bf16, kind="ExternalOutput").ap()
    flush_e = nc.dram_tensor("warm_flush", [P, 1], f32).ap()

    with tile.TileContext(nc) as tc:
        with (
            tc.tile_pool(name="consts", bufs=1) as consts,
            tc.tile_pool(name="bpool", bufs=1) as bpool,
            tc.tile_pool(name="xpool", bufs=1) as xpool,
            tc.tile_pool(name="hpool", bufs=1) as hpool,
            tc.tile_pool(name="pspool", bufs=6, space="PSUM") as pspool,
            tc.tile_pool(name="warmps", bufs=1, space="PSUM") as warmps,
        ):
            # PE warm-up during the input-DMA ramp (HAM clock gate).
            warm_sb = consts.tile([P, 256], bf16, tag="warm")
            nc.gpsimd.memset(warm_sb[:], 0.0)
            wps = warmps.tile([P, 256], f32)
            for i in range(N_WARM):
                nc.tensor.matmul(
                    wps[:],
                    warm_sb[:, 0:128],
                    warm_sb[:],
                    start=(i == 0),
                    stop=(i == N_WARM - 1),
                )
            flush_sb = consts.tile([P, 1], f32, tag="flush")
            nc.vector.tensor_copy(flush_sb[:], wps[:, 0:1])
            nc.gpsimd.dma_start(flush_e[:], flush_sb[:])

            # Input DMAs.  sync: x chunks (chunk 0 split in half so the first
            # accumulation can begin sooner).  scalar (ACT): av + b groups.
            av_sb = consts.tile([P, G], f32, tag="av")
            nc.scalar.dma_start(av_sb[:], av_e[:])

            x_sb = []
            for ni in range(NCHUNK):
                xtile = xpool.tile([P, KQ * CH], bf16, tag=f"x{ni}")
                base = ni * KQ * CH
                if ni == 0:
                    half = (KQ // 2) * CH
                    nc.sync.dma_start(xtile[:, 0:half], xt_e[:, base : base + half])
                    nc.sync.dma_start(
                        xtile[:, half : KQ * CH],
                        xt_e[:, base + half : base + KQ * CH],
                    )
                else:
                    nc.sync.dma_start(xtile[:], xt_e[:, base : base + KQ * CH])
                x_sb.append(xtile)

            b_sb = bpool.tile([P, G * H], bf16, tag="b")
            for g in range(G):
                nc.scalar.dma_start(
                    b_sb[:, g * H : (g + 1) * H], b_e[:, g * H : (g + 1) * H]
                )

            # a broadcast tiles on DVE (fast; ready before the first scan).
            a_bc = []
            ones = consts.tile([P, CH], f32, tag="ones")
            nc.vector.memset(ones[:], 1.0)
            for g in range(G):
                t = consts.tile([P, CH], f32, tag=f"abc{g}")
                nc.vector.tensor_scalar_mul(t[:], ones[:], av_sb[:, g : g + 1])
                a_bc.append(t)

            h_t = []
            for g in range(G):
                ht = hpool.tile([P, W], bf16, tag=f"h{g}")
                h_t.append(ht)

            for ni in range(NCHUNK):
                n0 = ni * CH
                for g in range(G):
                    ps = pspool.tile([P, CH], f32)
                    for kq in range(KQ):
                        nc.tensor.matmul(
                            ps[:],
                            b_sb[:, g * H + kq * P : g * H + (kq + 1) * P],
                            x_sb[ni][:, kq * CH : (kq + 1) * CH],
                            start=(kq == 0),
                            stop=(kq == KQ - 1),
                        )
                    init = 0.0 if ni == 0 else h_t[g][:, n0 - 1 : n0]
                    nc.vector.tensor_tensor_scan(
                        h_t[g][:, n0 : n0 + CH],
                        a_bc[g][:],
                        ps[:],
                        init,
                        op0=mybir.AluOpType.mult,
                        op1=mybir.AluOpType.add,
                    )
                    if ni == NCHUNK - 1:
                        nc.sync.dma_start(
                            out_e[:, g * T_LOC : (g + 1) * T_LOC],
                            h_t[g][:, HALO:W],
                        )

    nc.compile()
    return nc


def _get_nc():
    if "nc" not in _state:
        _state["nc"] = _build_nc()
    return _state["nc"]


def _shard_inputs(x_seq, a_diag, b_mat):
    x = np.asarray(x_seq, dtype=np.float32)
    a = np.asarray(a_diag, dtype=np.float32)
    b = np.asarray(b_mat, dtype=np.float32)

    x_pad = np.concatenate([np.zeros((HALO, H), np.float32), x], axis=0)
    xT = np.ascontiguousarray(x_pad.T).astype(BF16)  # [H, T + HALO]

    # b host layout: [P, g, kq, 128]: b_host[p, g*1024+kq*128+j] = b[kq*128+p, g*128+j]
    b_host = np.ascontiguousarray(
        b.reshape(KQ, P, G, P).transpose(1, 2, 0, 3).reshape(P, G * H)
    ).astype(BF16)
    av = np.ascontiguousarray(a.reshape(G, P).T)  # [P, G] fp32

    in_maps = []
    for i in range(NC):
        slab = xT[:, i * T_LOC : i * T_LOC + W]  # [H, W]
        sr = slab.reshape(KQ, P, W)
        # chunk-major: [P, ni, kq, CH]
        xt_host = np.concatenate(
            [
                sr[:, :, ni * CH : (ni + 1) * CH].transpose(1, 0, 2).reshape(P, -1)
                for ni in range(NCHUNK)
            ],
            axis=1,
        )
        in_maps.append(
            {
                "xt": np.ascontiguousarray(xt_host),
                "b": b_host,
                "av": av,
            }
        )
    return in_maps


def kernel(x_seq, a_diag, b_mat):
    from concourse.bass_utils import run_bass_kernel_spmd

    nc = _get_nc()
    in_maps = _shard_inputs(x_seq, a_diag, b_mat)
    res = run_bass_kernel_spmd(nc, in_maps, list(range(NC)))
    _state["last_result"] = res
    blocks = []
    for i in range(NC):
        o = np.asarray(res.results[i]["out"]).astype(np.float32)  # [P, G*T_LOC]
        # out[p, g*T_LOC + t] = h[t, g*128+p] for local t
        blocks.append(o.reshape(P, G, T_LOC).transpose(2, 1, 0).reshape(T_LOC, H))
    return np.concatenate(blocks, axis=0)
